# revision 7
# baseline (speedup 1.0000x reference)
"""Trainium2 Bass kernel for EncoderGRUODE (GRU-ODE encoder scan).

Reference semantics (per time step t, sequential over T=512):
    h_ode = rk4(h, dt_t)          # dh/dt = tanh(h @ W_node.T + b_node)
    prev  = h @ W_out.T + b_out
    inp   = x_t if mask_t else prev
    h     = GRUCell(inp, h_ode)   # torch GRUCell semantics
Output: stack(h over t) @ W_out.T + b_out, flattened to [B*T, D].

Mapping: data-parallel over batch, B=256 -> 8 cores x 32. Per core the
state lives transposed in SBUF as hT [H=128 partitions, 32 cols]; every
matmul loads a (host-pretransposed) weight as the stationary operand and
streams the narrow state. All matmul operands are bf16 (PE runs 4x the
fp32 rate); the recurrent state h, PSUM accumulation, and every
elementwise op stay fp32, so bf16 noise enters only through matmul
inputs. RK4 stage inputs (h + c*k) are never formed: the matmuls
accumulate W@h + (c*W)@k in PSUM with host-prescaled bf16 weight copies
(one set per distinct dt). The GRU gate matmuls are likewise
distributed, W_hh@h_ode = W_hh@h + (dt/6*W_hh)@S with S = k1+2k2+2k3+k4,
which moves the big W_hh@h work off the critical path (h is known at
step start). Gate biases enter PSUM through a tiny K<=2 outer-product
matmul so the r|z sigmoid is a single activation op. The scan is fully
unrolled (mask/dt are compile-time constants); the [B*T, D] projection
runs as 128 block matmuls at the end.
"""

import sys

sys.path.insert(0, "/opt/trn_rl_repo")

from contextlib import ExitStack  # noqa: E402

import ml_dtypes  # noqa: E402
import numpy as np  # noqa: E402

import concourse.bacc as bacc  # noqa: E402
import concourse.mybir as mybir  # noqa: E402
import concourse.tile as tile  # noqa: E402
from concourse.bass_utils import run_bass_kernel_spmd  # noqa: E402

B, T, D, H = 256, 512, 64, 128
NCORES = 8
BL = B // NCORES  # 32 batch rows per core
FP = mybir.dt.float32
BF = mybir.dt.bfloat16
NPBF = ml_dtypes.bfloat16
AF = mybir.ActivationFunctionType
OP = mybir.AluOpType


def build_program(dts, mask, n_steps, debug_h=False):
    """Trace + schedule + compile the per-core Bass program.

    dts/mask are baked in as compile-time constants (the kernel
    specializes per call; build is cached on their values).
    """
    dts = np.asarray(dts, np.float32)
    uniq = np.unique(dts)
    assert len(uniq) <= 32, f"too many distinct dts: {len(uniq)}"
    dt_idx = {float(v): i for i, v in enumerate(uniq)}
    nu = len(uniq)

    nc = bacc.Bacc("TRN2", target_bir_lowering=False, debug=False,
                   num_devices=NCORES)

    def din(name, shape, dt_=BF):
        return nc.dram_tensor(name, list(shape), dt_, kind="ExternalInput").ap()

    xT_d = din("xT", (D, BL, n_steps))    # xT[d, b, t] = x[b, t, d]
    wt_d = din("wt", (H, H))              # W_node.T
    wt_h_d = [din(f"wt_h{u}", (H, H)) for u in range(nu)]   # 0.5*dt*W^T
    wt_f_d = [din(f"wt_f{u}", (H, H)) for u in range(nu)]   # dt*W^T
    whh_d = [din(f"whh{g}", (H, H)) for g in range(3)]      # W_hh[g].T
    wh6_d = [[din(f"wh6_{u}_{g}", (H, H)) for g in range(3)]
             for u in range(nu)]                            # dt/6*W_hh[g].T
    wih_d = [din(f"wih{g}", (D, H)) for g in range(3)]      # W_ih[g].T
    wout_d = din("woutT", (H, D))         # W_out.T
    bias2_d = din("bias2", (2, H))        # rows: b_r, b_z (combined ih+hh)
    ind2_d = din("ind2", (2, 2 * BL))     # block indicator for r|z cols
    bhn_d = din("bhn", (1, H))            # b_hh_n row
    ones_bl_d = din("ones_bl", (1, BL))
    ones_p_d = din("ones_p", (1, H))
    bout_row_d = din("bout_row", (1, D))
    bnode_d = din("bnode", (H, 1), FP)
    bihn_d = din("bihn", (H, 1), FP)
    bout_v_d = din("bout_v", (D, 1), FP)
    out_d = nc.dram_tensor("out", [BL * n_steps, D], FP,
                           kind="ExternalOutput").ap()
    hdbg_d = (nc.dram_tensor("h_dbg", [H, BL, n_steps], FP,
                             kind="ExternalOutput").ap() if debug_h else None)

    with tile.TileContext(nc) as tc, ExitStack() as ctx:
        big = ctx.enter_context(tc.tile_pool(name="big", bufs=1))
        wpool = ctx.enter_context(tc.tile_pool(name="weights", bufs=1))
        work = ctx.enter_context(tc.tile_pool(name="work", bufs=2))

        # ---- persistent SBUF tensors -------------------------------------
        xT = big.tile([D, BL, n_steps], BF, name="xT", tag="xT")
        hT_all = big.tile([H, BL, n_steps], FP, name="hT_all", tag="hT_all")
        hT_all_bf = big.tile([H, BL, n_steps], BF, name="hT_all_bf",
                             tag="hT_all_bf")

        def wtile(name, shape, dt_=BF):
            return wpool.tile(list(shape), dt_, name=name, tag=name)

        wt = wtile("wt", (H, H))
        wt_h = [wtile(f"wt_h{u}", (H, H)) for u in range(nu)]
        wt_f = [wtile(f"wt_f{u}", (H, H)) for u in range(nu)]
        whh = [wtile(f"whh{g}", (H, H)) for g in range(3)]
        wh6 = [[wtile(f"wh6_{u}_{g}", (H, H)) for g in range(3)]
               for u in range(nu)]
        wih = [wtile(f"wih{g}", (D, H)) for g in range(3)]
        woutT = wtile("woutT", (H, D))
        bias2 = wtile("bias2", (2, H))
        ind2 = wtile("ind2", (2, 2 * BL))
        bhn = wtile("bhn", (1, H))
        ones_bl = wtile("ones_bl", (1, BL))
        ones_p = wtile("ones_p", (1, H))
        bout_row = wtile("bout_row", (1, D))
        bnode = wtile("bnode", (H, 1), FP)
        bihn = wtile("bihn", (H, 1), FP)
        bout_v = wtile("bout_v", (D, 1), FP)
        h0f = wtile("h0f", (H, BL), FP)
        h0b = wtile("h0b", (H, BL), BF)

        for t_sb, t_dr in [
            (xT, xT_d), (wt, wt_d), (woutT, wout_d), (bias2, bias2_d),
            (ind2, ind2_d), (bhn, bhn_d), (ones_bl, ones_bl_d),
            (ones_p, ones_p_d), (bout_row, bout_row_d), (bnode, bnode_d),
            (bihn, bihn_d), (bout_v, bout_v_d),
        ]:
            nc.sync.dma_start(t_sb[:], t_dr)
        for u in range(nu):
            nc.sync.dma_start(wt_h[u][:], wt_h_d[u])
            nc.sync.dma_start(wt_f[u][:], wt_f_d[u])
            for g in range(3):
                nc.sync.dma_start(wh6[u][g][:], wh6_d[u][g])
        for g in range(3):
            nc.sync.dma_start(whh[g][:], whh_d[g])
            nc.sync.dma_start(wih[g][:], wih_d[g])
        nc.vector.memset(h0f[:], 0.0)
        nc.vector.memset(h0b[:], 0.0)

        # ---- PSUM pools ---------------------------------------------------
        prk = ctx.enter_context(tc.tile_pool(name="prk", bufs=1, space="PSUM"))
        pg1 = ctx.enter_context(tc.tile_pool(name="pg1", bufs=2, space="PSUM"))
        pg2 = ctx.enter_context(tc.tile_pool(name="pg2", bufs=2, space="PSUM"))
        ppv = ctx.enter_context(tc.tile_pool(name="ppv", bufs=1, space="PSUM"))

        # ---- the scan -----------------------------------------------------
        for t_ in range(n_steps):
            dt = float(dts[t_])
            u = dt_idx[dt]
            m_t = bool(mask[t_])
            if t_ == 0:
                hTf, hTb = h0f[:], h0b[:]
            else:
                hTf = hT_all[:, :, t_ - 1]
                hTb = hT_all_bf[:, :, t_ - 1]

            # gate PSUM tiles: g1 = [r | z], g2 = [h_n | i_n]
            g1 = pg1.tile([H, 2 * BL], FP, name="g1", tag="g1")
            g2 = pg2.tile([H, 2 * BL], FP, name="g2", tag="g2")
            # bias prefill via K<=2 outer-product matmul (start=True clears
            # the whole bank; everything after accumulates with start=False)
            nc.tensor.matmul(g1[:], bias2[:], ind2[:], start=True, stop=False,
                             skip_group_check=True)
            nc.tensor.matmul(g2[:, 0:BL], bhn[:], ones_bl[:], start=True,
                             stop=False, skip_group_check=True)

            # W_hh @ h contribution (h known at step start -> off crit path)
            nc.tensor.matmul(g1[:, 0:BL], whh[0][:], hTb, start=False,
                             stop=False, skip_group_check=True)
            nc.tensor.matmul(g1[:, BL:2 * BL], whh[1][:], hTb, start=False,
                             stop=False, skip_group_check=True)
            nc.tensor.matmul(g2[:, 0:BL], whh[2][:], hTb, start=False,
                             stop=False, skip_group_check=True)

            # input vector (transposed): x_t column gather or prev_out
            if m_t:
                inpT = xT[:, :, t_]
            else:
                ppv_t = ppv.tile([D, BL], FP, name="pprev", tag="pprev")
                nc.tensor.matmul(ppv_t[:], woutT[:], hTb, start=True,
                                 stop=True)
                inp_sb = work.tile([D, BL], BF, name="inpT", tag="inpT")
                nc.scalar.activation(inp_sb[:], ppv_t[:], AF.Identity,
                                     bias=bout_v[:])
                inpT = inp_sb[:]

            # W_ih contributions (off the critical path)
            nc.tensor.matmul(g1[:, 0:BL], wih[0][:], inpT, start=False,
                             stop=False, skip_group_check=True)
            nc.tensor.matmul(g1[:, BL:2 * BL], wih[1][:], inpT, start=False,
                             stop=False, skip_group_check=True)
            nc.tensor.matmul(g2[:, BL:2 * BL], wih[2][:], inpT, start=False,
                             stop=True, skip_group_check=True)

            # ---- RK4: psum bank [k1 | k2 | k3 | k4] ----
            rk = prk.tile([H, 4 * BL], FP, name="rk", tag="rk")
            ks = [work.tile([H, BL], BF, name=f"k{i}", tag=f"k{i}")
                  for i in range(4)]
            nc.tensor.matmul(rk[:, 0:BL], wt[:], hTb, start=True, stop=True,
                             skip_group_check=True)
            nc.scalar.activation(ks[0][:], rk[:, 0:BL], AF.Tanh, bias=bnode[:])
            for i, (wk, k_prev) in enumerate(
                    [(wt_h[u], ks[0]), (wt_h[u], ks[1]), (wt_f[u], ks[2])]):
                sl = rk[:, (i + 1) * BL:(i + 2) * BL]
                nc.tensor.matmul(sl, wt[:], hTb, start=False, stop=False,
                                 skip_group_check=True)
                nc.tensor.matmul(sl, wk[:], k_prev[:], start=False, stop=True,
                                 skip_group_check=True)
                nc.scalar.activation(ks[i + 1][:], sl, AF.Tanh, bias=bnode[:])

            # S = k1 + 2*(k2+k3) + k4  (bf16; only aa,S are on the crit path)
            uu = work.tile([H, BL], BF, name="uu", tag="uu")
            nc.vector.tensor_tensor(uu[:], ks[1][:], ks[2][:], op=OP.add)
            b2 = work.tile([H, BL], BF, name="b2", tag="b2")
            nc.vector.tensor_tensor(b2[:], uu[:], uu[:], op=OP.add)
            aa = work.tile([H, BL], BF, name="aa", tag="aa")
            nc.vector.tensor_tensor(aa[:], ks[0][:], ks[3][:], op=OP.add)
            S = work.tile([H, BL], BF, name="S", tag="S")
            nc.vector.tensor_tensor(S[:], aa[:], b2[:], op=OP.add)

            # h_ode = h + (dt/6)*S   (fp32, shadow: consumers are DVE-only)
            cc = work.tile([H, BL], FP, name="cc", tag="cc")
            nc.vector.tensor_scalar_mul(cc[:], S[:], dt / 6.0)
            hode = work.tile([H, BL], FP, name="hode", tag="hode")
            nc.vector.tensor_tensor(hode[:], hTf, cc[:], op=OP.add)

            # ---- GRU gates: += (dt/6 * W_hh) @ S  (critical part) ----
            nc.tensor.matmul(g1[:, 0:BL], wh6[u][0][:], S[:], start=False,
                             stop=True, skip_group_check=True)
            nc.tensor.matmul(g1[:, BL:2 * BL], wh6[u][1][:], S[:],
                             start=False, stop=True, skip_group_check=True)
            nc.tensor.matmul(g2[:, 0:BL], wh6[u][2][:], S[:], start=False,
                             stop=True, skip_group_check=True)

            rz = work.tile([H, 2 * BL], FP, name="rz", tag="rz")
            nc.scalar.activation(rz[:], g1[:], AF.Sigmoid)
            mm_ = work.tile([H, BL], FP, name="mm_", tag="mm_")
            nc.vector.tensor_tensor(mm_[:], rz[:, 0:BL], g2[:, 0:BL],
                                    op=OP.mult)
            ss = work.tile([H, BL], FP, name="ss", tag="ss")
            nc.vector.tensor_tensor(ss[:], mm_[:], g2[:, BL:2 * BL], op=OP.add)
            nT = work.tile([H, BL], FP, name="nT", tag="nT")
            nc.scalar.activation(nT[:], ss[:], AF.Tanh, bias=bihn[:])

            omz = work.tile([H, BL], FP, name="omz", tag="omz")
            nc.vector.tensor_scalar(omz[:], rz[:, BL:2 * BL], -1.0, 1.0,
                                    op0=OP.mult, op1=OP.add)
            zh = work.tile([H, BL], FP, name="zh", tag="zh")
            nc.vector.tensor_tensor(zh[:], rz[:, BL:2 * BL], hode[:],
                                    op=OP.mult)
            t1 = work.tile([H, BL], FP, name="t1", tag="t1")
            nc.vector.tensor_tensor(t1[:], nT[:], omz[:], op=OP.mult)
            # bf16 h_new first (feeds next step's matmuls), fp32 in shadow
            nc.vector.tensor_tensor(hT_all_bf[:, :, t_], t1[:], zh[:],
                                    op=OP.add)
            nc.vector.tensor_tensor(hT_all[:, :, t_], t1[:], zh[:], op=OP.add)

        if debug_h:
            nc.sync.dma_start(hdbg_d, hT_all[:])

        # ---- output projection: out[b*T+t, :] = h(b,t) @ W_out.T + b_out --
        with tc.tile_pool(name="pproj", bufs=2, space="PSUM") as pproj, \
                tc.tile_pool(name="oproj", bufs=4) as oproj:
            for b_ in range(BL):
                for tq in range(0, n_steps, H):
                    w_blk = min(H, n_steps - tq)
                    po = pproj.tile([H, D], FP, name="po", tag="po")
                    nc.tensor.matmul(po[0:w_blk, :],
                                     hT_all_bf[:, b_, tq:tq + w_blk],
                                     woutT[:], start=True, stop=False,
                                     skip_group_check=True)
                    nc.tensor.matmul(po[0:w_blk, :], ones_p[:, 0:w_blk],
                                     bout_row[:], start=False, stop=True,
                                     skip_group_check=True)
                    ob = oproj.tile([H, D], FP, name="ob", tag="ob")
                    nc.scalar.copy(ob[0:w_blk, :], po[0:w_blk, :])
                    r0 = b_ * n_steps + tq
                    nc.sync.dma_start(out_d[r0:r0 + w_blk, :], ob[0:w_blk, :])

    nc.compile()
    return nc


_CACHE = {}


def _get_program(dts, mask, n_steps):
    key = (dts.tobytes(), mask.tobytes(), n_steps)
    if key not in _CACHE:
        _CACHE[key] = build_program(dts, mask, n_steps)
    return _CACHE[key]


def prepare_host(inputs, n_steps=T):
    """Host-side prep shared by kernel() and the test harness."""
    x = np.ascontiguousarray(np.asarray(inputs["x"], np.float32))
    tp = np.asarray(inputs["tp"], np.float32)
    mask = np.asarray(inputs["samp_mask"]).astype(bool)[:n_steps]
    W_ih = np.asarray(inputs["W_ih"], np.float32)
    W_hh = np.asarray(inputs["W_hh"], np.float32)
    b_ih = np.asarray(inputs["b_ih"], np.float32)
    b_hh = np.asarray(inputs["b_hh"], np.float32)
    W_node = np.asarray(inputs["W_node"], np.float32)
    b_node = np.asarray(inputs["b_node"], np.float32)
    W_out = np.asarray(inputs["W_out"], np.float32)
    b_out = np.asarray(inputs["b_out"], np.float32)

    t0 = tp[0]
    ts_ = np.concatenate([t0[:1] - np.float32(0.01), t0])
    dts = (ts_[1:] - ts_[:-1]).astype(np.float32)[:n_steps]
    uniq = np.unique(dts)

    bf = lambda a: np.ascontiguousarray(np.asarray(a, np.float32)).astype(NPBF)
    shared = {
        "wt": bf(W_node.T),
        "woutT": bf(W_out.T),
        "bias2": bf(np.stack([b_ih[0:H] + b_hh[0:H],
                              b_ih[H:2 * H] + b_hh[H:2 * H]])),
        "ind2": bf(np.concatenate(
            [np.concatenate([np.ones((1, BL), np.float32),
                             np.zeros((1, BL), np.float32)], 1),
             np.concatenate([np.zeros((1, BL), np.float32),
                             np.ones((1, BL), np.float32)], 1)], 0)),
        "bhn": bf(b_hh[2 * H:3 * H].reshape(1, H)),
        "ones_bl": bf(np.ones((1, BL), np.float32)),
        "ones_p": bf(np.ones((1, H), np.float32)),
        "bout_row": bf(b_out.reshape(1, D)),
        "bnode": b_node.reshape(H, 1).copy(),
        "bihn": b_ih[2 * H:3 * H].reshape(H, 1).copy(),
        "bout_v": b_out.reshape(D, 1).copy(),
    }
    for u, dv in enumerate(uniq):
        dv = np.float32(dv)
        shared[f"wt_h{u}"] = bf((np.float32(0.5) * dv) * W_node.T)
        shared[f"wt_f{u}"] = bf(dv * W_node.T)
        for g in range(3):
            shared[f"wh6_{u}_{g}"] = bf(
                (dv / np.float32(6.0)) * W_hh[g * H:(g + 1) * H].T)
    for g in range(3):
        shared[f"whh{g}"] = bf(W_hh[g * H:(g + 1) * H].T)
        shared[f"wih{g}"] = bf(W_ih[g * H:(g + 1) * H].T)

    in_maps = []
    for c in range(NCORES):
        xc = x[c * BL:(c + 1) * BL, :n_steps, :]           # [BL, n, D]
        mcore = dict(shared)
        mcore["xT"] = bf(xc.transpose(2, 0, 1))            # [D, BL, n]
        in_maps.append(mcore)
    return dts, mask, in_maps


def kernel(**inputs):
    dts, mask, in_maps = prepare_host(inputs, T)
    nc = _get_program(dts, mask, T)
    res = run_bass_kernel_spmd(nc, in_maps, list(range(NCORES)))
    outs = [np.asarray(res.results[c]["out"], np.float32)
            for c in range(NCORES)]
    return np.concatenate(outs, axis=0)


# revision 9
# speedup vs baseline: 1.1402x; 1.1402x over previous
"""Trainium2 Bass kernel for EncoderGRUODE (GRU-ODE encoder scan).

Reference semantics (per time step t, sequential over T=512):
    h_ode = rk4(h, dt_t)          # dh/dt = tanh(h @ W_node.T + b_node)
    prev  = h @ W_out.T + b_out
    inp   = x_t if mask_t else prev
    h     = GRUCell(inp, h_ode)   # torch GRUCell semantics
Output: stack(h over t) @ W_out.T + b_out, flattened to [B*T, D].

Mapping: data-parallel over batch, B=256 -> 8 cores x 32. Per core the
state lives transposed in SBUF as hT [H=128 partitions, 32 cols]; every
matmul loads a (host-pretransposed) weight as the stationary operand and
streams the narrow state. The 512-step scan is latency-bound, so the
structure minimizes the serial chain:
  * all matmul operands are fp16 (PE runs at 4x the fp32 rate; fp16
    keeps ~5e-4 relative precision vs bf16's 4e-3); recurrent state h,
    PSUM accumulation and elementwise ops stay fp32
  * RK4 stage inputs (h + c*k) are never formed: PSUM accumulates
    W@h + (c*W)@k with host-prescaled weight copies per distinct dt
  * GRU gate matmuls distribute W_hh@h_ode as W_hh@h (issued at step
    start, off the critical path) + (dt/6*W_hh)@{k1, 2(k2+k3), k4}
    streams, so only the k4 stream is critical
  * the next step's W@h matmul is fed by {W@zh, W@t1} before h itself
    is assembled (h = t1 + zh), removing the h-assembly from the chain
  * 1-z is produced on the Scalar engine as sigmoid(-x), fp32 h
    bookkeeping runs on GPSIMD, keeping the DVE free for the chain
  * gate biases enter PSUM via a K<=2 outer-product matmul so the r|z
    sigmoid is a single activation op
The scan is fully unrolled (mask/dt are compile-time constants); the
[B*T, D] output projection is interleaved into the scan's idle engine
slots, with the last quarter after the scan.
"""

import sys

sys.path.insert(0, "/opt/trn_rl_repo")

from contextlib import ExitStack  # noqa: E402

import numpy as np  # noqa: E402

import concourse.bacc as bacc  # noqa: E402
import concourse.mybir as mybir  # noqa: E402
import concourse.tile as tile  # noqa: E402
from concourse.bass_utils import run_bass_kernel_spmd  # noqa: E402

B, T, D, H = 256, 512, 64, 128
NCORES = 8
BL = B // NCORES  # 32 batch rows per core
FP = mybir.dt.float32
HF = mybir.dt.float16
AF = mybir.ActivationFunctionType
OP = mybir.AluOpType


def build_program(dts, mask, n_steps, debug_h=False):
    dts = np.asarray(dts, np.float32)
    uniq = np.unique(dts)
    assert len(uniq) <= 32, f"too many distinct dts: {len(uniq)}"
    dt_idx = {float(v): i for i, v in enumerate(uniq)}
    nu = len(uniq)

    nc = bacc.Bacc("TRN2", target_bir_lowering=False, debug=False,
                   num_devices=NCORES)

    def din(name, shape, dt_=HF):
        return nc.dram_tensor(name, list(shape), dt_, kind="ExternalInput").ap()

    xT_d = din("xT", (D, BL, n_steps))    # xT[d, b, t] = x[b, t, d]
    wt_d = din("wt", (H, H))              # W_node.T
    wt_h_d = [din(f"wt_h{u}", (H, H)) for u in range(nu)]   # 0.5*dt*W^T
    wt_f_d = [din(f"wt_f{u}", (H, H)) for u in range(nu)]   # dt*W^T
    whh_d = [din(f"whh{g}", (H, H)) for g in range(3)]      # W_hh[g].T
    wh6_d = [[din(f"wh6_{u}_{g}", (H, H)) for g in range(3)]
             for u in range(nu)]                            # dt/6*W_hh[g].T
    wih_d = [din(f"wih{g}", (D, H)) for g in range(3)]      # W_ih[g].T
    wout_d = din("woutT", (H, D))         # W_out.T
    bias2_d = din("bias2", (2, H))        # rows: b_r, b_z (combined ih+hh)
    ind2_d = din("ind2", (2, 2 * BL))     # block indicator for r|z cols
    bhn_d = din("bhn", (1, H))            # b_hh_n row
    ones_bl_d = din("ones_bl", (1, BL))
    ones_p_d = din("ones_p", (1, H))
    bout_row_d = din("bout_row", (1, D))
    bnode_d = din("bnode", (H, 1), FP)
    bihn_d = din("bihn", (H, 1), FP)
    bout_v_d = din("bout_v", (D, 1), FP)
    out_d = nc.dram_tensor("out", [BL * n_steps, D], FP,
                           kind="ExternalOutput").ap()
    hdbg_d = (nc.dram_tensor("h_dbg", [H, BL, n_steps], FP,
                             kind="ExternalOutput").ap() if debug_h else None)

    with tile.TileContext(nc) as tc, ExitStack() as ctx:
        big = ctx.enter_context(tc.tile_pool(name="big", bufs=1))
        wpool = ctx.enter_context(tc.tile_pool(name="weights", bufs=1))
        work = ctx.enter_context(tc.tile_pool(name="work", bufs=2))

        xT = big.tile([D, BL, n_steps], HF, name="xT", tag="xT")
        hT_all_h = big.tile([H, BL, n_steps], HF, name="hT_all_h",
                            tag="hT_all_h")
        hdbg = (big.tile([H, BL, n_steps], FP, name="hdbg", tag="hdbg")
                if debug_h else None)

        def wtile(name, shape, dt_=HF):
            return wpool.tile(list(shape), dt_, name=name, tag=name)

        wt = wtile("wt", (H, H))
        wt_h = [wtile(f"wt_h{u}", (H, H)) for u in range(nu)]
        wt_f = [wtile(f"wt_f{u}", (H, H)) for u in range(nu)]
        whh = [wtile(f"whh{g}", (H, H)) for g in range(3)]
        wh6 = [[wtile(f"wh6_{u}_{g}", (H, H)) for g in range(3)]
               for u in range(nu)]
        wih = [wtile(f"wih{g}", (D, H)) for g in range(3)]
        woutT = wtile("woutT", (H, D))
        bias2 = wtile("bias2", (2, H))
        ind2 = wtile("ind2", (2, 2 * BL))
        bhn = wtile("bhn", (1, H))
        ones_bl = wtile("ones_bl", (1, BL))
        ones_p = wtile("ones_p", (1, H))
        bout_row = wtile("bout_row", (1, D))
        bnode = wtile("bnode", (H, 1), FP)
        bihn = wtile("bihn", (H, 1), FP)
        bout_v = wtile("bout_v", (D, 1), FP)
        h0f = wtile("h0f", (H, BL), FP)
        h0h = wtile("h0h", (H, BL), HF)

        for t_sb, t_dr in [
            (xT, xT_d), (wt, wt_d), (woutT, wout_d), (bias2, bias2_d),
            (ind2, ind2_d), (bhn, bhn_d), (ones_bl, ones_bl_d),
            (ones_p, ones_p_d), (bout_row, bout_row_d), (bnode, bnode_d),
            (bihn, bihn_d), (bout_v, bout_v_d),
        ]:
            nc.sync.dma_start(t_sb[:], t_dr)
        for u in range(nu):
            nc.sync.dma_start(wt_h[u][:], wt_h_d[u])
            nc.sync.dma_start(wt_f[u][:], wt_f_d[u])
            for g in range(3):
                nc.sync.dma_start(wh6[u][g][:], wh6_d[u][g])
        for g in range(3):
            nc.sync.dma_start(whh[g][:], whh_d[g])
            nc.sync.dma_start(wih[g][:], wih_d[g])
        nc.vector.memset(h0f[:], 0.0)
        nc.vector.memset(h0h[:], 0.0)

        prk = ctx.enter_context(tc.tile_pool(name="prk", bufs=1, space="PSUM"))
        pg1 = ctx.enter_context(tc.tile_pool(name="pg1", bufs=2, space="PSUM"))
        pg2 = ctx.enter_context(tc.tile_pool(name="pg2", bufs=2, space="PSUM"))
        ppv = ctx.enter_context(tc.tile_pool(name="ppv", bufs=1, space="PSUM"))
        ppj = ctx.enter_context(tc.tile_pool(name="ppj", bufs=2, space="PSUM"))
        opj = ctx.enter_context(tc.tile_pool(name="opj", bufs=4))

        hprev_f = [wtile("hprev_f0", (H, BL), FP),
                   wtile("hprev_f1", (H, BL), FP)]

        def emit_proj_block(i):
            """Project block i (b = i%BL, tq = i//BL) -> out rows."""
            tq, b_ = divmod(i, BL)
            c0 = tq * H
            w_blk = min(H, n_steps - c0)
            po = ppj.tile([H, D], FP, name="po", tag="po")
            nc.tensor.matmul(po[0:w_blk, :], hT_all_h[:, b_, c0:c0 + w_blk],
                             woutT[:], start=True, stop=False,
                             skip_group_check=True)
            nc.tensor.matmul(po[0:w_blk, :], ones_p[:, 0:w_blk], bout_row[:],
                             start=False, stop=True, skip_group_check=True)
            ob = opj.tile([H, D], FP, name="ob", tag="ob")
            nc.scalar.copy(ob[0:w_blk, :], po[0:w_blk, :])
            r0 = b_ * n_steps + c0
            nc.sync.dma_start(out_d[r0:r0 + w_blk, :], ob[0:w_blk, :])

        n_blocks = BL * ((n_steps + H - 1) // H)
        next_block = 0

        zh_h_prev = t1_h_prev = None
        for t_ in range(n_steps):
            dt = float(dts[t_])
            u = dt_idx[dt]
            m_t = bool(mask[t_])
            hTf = h0f[:] if t_ == 0 else hprev_f[(t_ - 1) % 2][:]
            hTh = h0h[:] if t_ == 0 else hT_all_h[:, :, t_ - 1]

            # ---- RK4 bank [k1 | k2 | k3 | k4]; k1 region first (crit) ----
            rk = prk.tile([H, 4 * BL], FP, name="rk", tag="rk")
            if t_ == 0:
                nc.tensor.matmul(rk[:, 0:BL], wt[:], hTh, start=True,
                                 stop=True, skip_group_check=True)
            else:
                # W@h = W@zh + W@t1, issued before h itself exists
                nc.tensor.matmul(rk[:, 0:BL], wt[:], zh_h_prev[:], start=True,
                                 stop=False, skip_group_check=True)
                nc.tensor.matmul(rk[:, 0:BL], wt[:], t1_h_prev[:],
                                 start=False, stop=True, skip_group_check=True)
            for i in range(3):
                nc.tensor.matmul(rk[:, (i + 1) * BL:(i + 2) * BL], wt[:], hTh,
                                 start=False, stop=False,
                                 skip_group_check=True)
            ks = [work.tile([H, BL], HF, name=f"k{i}", tag=f"k{i}")
                  for i in range(4)]
            nc.scalar.activation(ks[0][:], rk[:, 0:BL], AF.Tanh, bias=bnode[:])

            # ---- gate banks: g1 = [r | z], g2 = [h_n | i_n] ----
            g1 = pg1.tile([H, 2 * BL], FP, name="g1", tag="g1")
            g2 = pg2.tile([H, 2 * BL], FP, name="g2", tag="g2")
            nc.tensor.matmul(g1[:], bias2[:], ind2[:], start=True, stop=False,
                             skip_group_check=True)
            nc.tensor.matmul(g2[:, 0:BL], bhn[:], ones_bl[:], start=True,
                             stop=False, skip_group_check=True)
            nc.tensor.matmul(g1[:, 0:BL], whh[0][:], hTh, start=False,
                             stop=False, skip_group_check=True)
            nc.tensor.matmul(g1[:, BL:2 * BL], whh[1][:], hTh, start=False,
                             stop=False, skip_group_check=True)
            nc.tensor.matmul(g2[:, 0:BL], whh[2][:], hTh, start=False,
                             stop=False, skip_group_check=True)
            # (dt/6*W_hh) @ k1 as soon as k1 exists
            nc.tensor.matmul(g1[:, 0:BL], wh6[u][0][:], ks[0][:], start=False,
                             stop=False, skip_group_check=True)
            nc.tensor.matmul(g1[:, BL:2 * BL], wh6[u][1][:], ks[0][:],
                             start=False, stop=False, skip_group_check=True)
            nc.tensor.matmul(g2[:, 0:BL], wh6[u][2][:], ks[0][:], start=False,
                             stop=False, skip_group_check=True)

            # input vector: x_t column gather or prev_out
            if m_t:
                inpT = xT[:, :, t_]
            else:
                ppv_t = ppv.tile([D, BL], FP, name="pprev", tag="pprev")
                nc.tensor.matmul(ppv_t[:], woutT[:], hTh, start=True,
                                 stop=True)
                inp_sb = work.tile([D, BL], HF, name="inpT", tag="inpT")
                nc.scalar.activation(inp_sb[:], ppv_t[:], AF.Identity,
                                     bias=bout_v[:])
                inpT = inp_sb[:]
            nc.tensor.matmul(g1[:, 0:BL], wih[0][:], inpT, start=False,
                             stop=False, skip_group_check=True)
            nc.tensor.matmul(g1[:, BL:2 * BL], wih[1][:], inpT, start=False,
                             stop=False, skip_group_check=True)
            nc.tensor.matmul(g2[:, BL:2 * BL], wih[2][:], inpT, start=False,
                             stop=True, skip_group_check=True)

            # ---- RK4 accumulation rounds (critical) ----
            for i, (wk, k_prev) in enumerate(
                    [(wt_h[u], ks[0]), (wt_h[u], ks[1]), (wt_f[u], ks[2])]):
                sl = rk[:, (i + 1) * BL:(i + 2) * BL]
                nc.tensor.matmul(sl, wk[:], k_prev[:], start=False, stop=True,
                                 skip_group_check=True)
                nc.scalar.activation(ks[i + 1][:], sl, AF.Tanh, bias=bnode[:])

            # b2 = 2*(k2+k3) stream (ready before k4's tanh finishes)
            uu = work.tile([H, BL], HF, name="uu", tag="uu")
            nc.vector.tensor_tensor(uu[:], ks[1][:], ks[2][:], op=OP.add)
            b2 = work.tile([H, BL], HF, name="b2", tag="b2")
            nc.vector.tensor_tensor(b2[:], uu[:], uu[:], op=OP.add)
            nc.tensor.matmul(g1[:, 0:BL], wh6[u][0][:], b2[:], start=False,
                             stop=False, skip_group_check=True)
            nc.tensor.matmul(g1[:, BL:2 * BL], wh6[u][1][:], b2[:],
                             start=False, stop=False, skip_group_check=True)
            nc.tensor.matmul(g2[:, 0:BL], wh6[u][2][:], b2[:], start=False,
                             stop=False, skip_group_check=True)
            # k4 stream (critical)
            nc.tensor.matmul(g1[:, 0:BL], wh6[u][0][:], ks[3][:], start=False,
                             stop=True, skip_group_check=True)
            nc.tensor.matmul(g1[:, BL:2 * BL], wh6[u][1][:], ks[3][:],
                             start=False, stop=True, skip_group_check=True)
            nc.tensor.matmul(g2[:, 0:BL], wh6[u][2][:], ks[3][:], start=False,
                             stop=True, skip_group_check=True)

            # h_ode = h + (dt/6)*S (fp32; consumers are DVE/GPSIMD only)
            aa = work.tile([H, BL], HF, name="aa", tag="aa")
            nc.vector.tensor_tensor(aa[:], ks[0][:], ks[3][:], op=OP.add)
            S = work.tile([H, BL], HF, name="S", tag="S")
            nc.vector.tensor_tensor(S[:], aa[:], b2[:], op=OP.add)
            cc = work.tile([H, BL], FP, name="cc", tag="cc")
            nc.vector.tensor_scalar_mul(cc[:], S[:], dt / 6.0)
            hode = work.tile([H, BL], FP, name="hode", tag="hode")
            nc.vector.tensor_tensor(hode[:], hTf, cc[:], op=OP.add)

            # ---- gates ----
            rz = work.tile([H, 2 * BL], FP, name="rz", tag="rz")
            nc.scalar.activation(rz[:], g1[:], AF.Sigmoid)
            omz = work.tile([H, BL], FP, name="omz", tag="omz")
            nc.scalar.activation(omz[:], g1[:, BL:2 * BL], AF.Sigmoid,
                                 scale=-1.0)
            mm_ = work.tile([H, BL], FP, name="mm_", tag="mm_")
            nc.vector.tensor_tensor(mm_[:], rz[:, 0:BL], g2[:, 0:BL],
                                    op=OP.mult)
            ss = work.tile([H, BL], FP, name="ss", tag="ss")
            nc.vector.tensor_tensor(ss[:], mm_[:], g2[:, BL:2 * BL], op=OP.add)
            nT = work.tile([H, BL], FP, name="nT", tag="nT")
            nc.scalar.activation(nT[:], ss[:], AF.Tanh, bias=bihn[:])

            # zh on GPSIMD (frees the DVE); fp16 copy feeds next step's mm
            zh_h = work.tile([H, BL], HF, name="zh_h", tag="zh_h")
            nc.gpsimd.tensor_tensor(zh_h[:], rz[:, BL:2 * BL], hode[:],
                                    op=OP.mult)
            zh_f = work.tile([H, BL], FP, name="zh_f", tag="zh_f")
            nc.gpsimd.tensor_tensor(zh_f[:], rz[:, BL:2 * BL], hode[:],
                                    op=OP.mult)
            # t1 = n*(1-z): fp16 on DVE (critical), fp32 on GPSIMD
            t1_h = work.tile([H, BL], HF, name="t1_h", tag="t1_h")
            nc.vector.tensor_tensor(t1_h[:], nT[:], omz[:], op=OP.mult)
            t1_f = work.tile([H, BL], FP, name="t1_f", tag="t1_f")
            nc.gpsimd.tensor_tensor(t1_f[:], nT[:], omz[:], op=OP.mult)
            # h (fp16 stream) = t1_h + zh_h; h (fp32) = t1_f + zh_f
            nc.vector.tensor_tensor(hT_all_h[:, :, t_], t1_h[:], zh_h[:],
                                    op=OP.add)
            hp = hprev_f[t_ % 2]
            nc.gpsimd.tensor_tensor(hp[:], t1_f[:], zh_f[:], op=OP.add)
            if debug_h:
                nc.vector.tensor_copy(hdbg[:, :, t_], hp[:])
            zh_h_prev, t1_h_prev = zh_h, t1_h

            # interleave output projection into engine idle slots
            if t_ >= H + 2 and (t_ - H - 2) % 3 == 0 and next_block < n_blocks:
                tq = next_block // BL
                if (tq + 1) * H <= t_:
                    emit_proj_block(next_block)
                    next_block += 1

        for i in range(next_block, n_blocks):
            emit_proj_block(i)

        if debug_h:
            nc.sync.dma_start(hdbg_d, hdbg[:])

    nc.compile()
    return nc


_CACHE = {}


def _get_program(dts, mask, n_steps):
    key = (dts.tobytes(), mask.tobytes(), n_steps)
    if key not in _CACHE:
        _CACHE[key] = build_program(dts, mask, n_steps)
    return _CACHE[key]


def prepare_host(inputs, n_steps=T):
    """Host-side prep shared by kernel() and the test harness."""
    x = np.ascontiguousarray(np.asarray(inputs["x"], np.float32))
    tp = np.asarray(inputs["tp"], np.float32)
    mask = np.asarray(inputs["samp_mask"]).astype(bool)[:n_steps]
    W_ih = np.asarray(inputs["W_ih"], np.float32)
    W_hh = np.asarray(inputs["W_hh"], np.float32)
    b_ih = np.asarray(inputs["b_ih"], np.float32)
    b_hh = np.asarray(inputs["b_hh"], np.float32)
    W_node = np.asarray(inputs["W_node"], np.float32)
    b_node = np.asarray(inputs["b_node"], np.float32)
    W_out = np.asarray(inputs["W_out"], np.float32)
    b_out = np.asarray(inputs["b_out"], np.float32)

    t0 = tp[0]
    ts_ = np.concatenate([t0[:1] - np.float32(0.01), t0])
    dts = (ts_[1:] - ts_[:-1]).astype(np.float32)[:n_steps]
    uniq = np.unique(dts)

    hf = lambda a: np.ascontiguousarray(np.asarray(a, np.float32)).astype(
        np.float16)
    shared = {
        "wt": hf(W_node.T),
        "woutT": hf(W_out.T),
        "bias2": hf(np.stack([b_ih[0:H] + b_hh[0:H],
                              b_ih[H:2 * H] + b_hh[H:2 * H]])),
        "ind2": hf(np.concatenate(
            [np.concatenate([np.ones((1, BL), np.float32),
                             np.zeros((1, BL), np.float32)], 1),
             np.concatenate([np.zeros((1, BL), np.float32),
                             np.ones((1, BL), np.float32)], 1)], 0)),
        "bhn": hf(b_hh[2 * H:3 * H].reshape(1, H)),
        "ones_bl": hf(np.ones((1, BL), np.float32)),
        "ones_p": hf(np.ones((1, H), np.float32)),
        "bout_row": hf(b_out.reshape(1, D)),
        "bnode": b_node.reshape(H, 1).copy(),
        "bihn": b_ih[2 * H:3 * H].reshape(H, 1).copy(),
        "bout_v": b_out.reshape(D, 1).copy(),
    }
    for u, dv in enumerate(uniq):
        dv = np.float32(dv)
        shared[f"wt_h{u}"] = hf((np.float32(0.5) * dv) * W_node.T)
        shared[f"wt_f{u}"] = hf(dv * W_node.T)
        for g in range(3):
            shared[f"wh6_{u}_{g}"] = hf(
                (dv / np.float32(6.0)) * W_hh[g * H:(g + 1) * H].T)
    for g in range(3):
        shared[f"whh{g}"] = hf(W_hh[g * H:(g + 1) * H].T)
        shared[f"wih{g}"] = hf(W_ih[g * H:(g + 1) * H].T)

    in_maps = []
    for c in range(NCORES):
        xc = x[c * BL:(c + 1) * BL, :n_steps, :]           # [BL, n, D]
        mcore = dict(shared)
        mcore["xT"] = hf(xc.transpose(2, 0, 1))            # [D, BL, n]
        in_maps.append(mcore)
    return dts, mask, in_maps


def kernel(**inputs):
    dts, mask, in_maps = prepare_host(inputs, T)
    nc = _get_program(dts, mask, T)
    res = run_bass_kernel_spmd(nc, in_maps, list(range(NCORES)))
    outs = [np.asarray(res.results[c]["out"], np.float32)
            for c in range(NCORES)]
    return np.concatenate(outs, axis=0)
